# revision 29
# baseline (speedup 1.0000x reference)
"""AssistedExcitation Trainium2 kernel.

out[b,c,h,w] = x[b,c,h,w] + bbox_mask[b,h,w] * mean_c(x[b,:,h,w])

Data-parallel over 8 NeuronCores: 2 images per core, no collectives.
The problem is HBM-bandwidth bound (~360 GB/s/core aggregate DMA), so
x is shipped to the device and the output returned as bfloat16: the
2e-2 rel-err budget dwarfs bf16 quantization (~2e-3 measured) and it
halves DMA bytes vs f32.

Per core, per [256, 2048] chunk (channels on partitions, bf16):
 - channel-sum via PE matmul; a per-sub stationary column (wsum16)
   places sub s's sum on PSUM PARTITION s, so the [4,512] mask*mean
   multiply runs 4-lane on DVE instead of 1-lane (PSUM f32 pins DVE
   to 1 col/cycle; keep those ops small).
 - mask rasterized on device via outer-product matmuls (prologue).
 - broadcast across channels: K=4 matmul with a row-selector
   stationary into a [128,1024] PSUM pair; Activation engine evicts
   to bf16 SBUF so the final DVE adds run all-bf16-SBUF at 2 cols/
   cycle; stores from the scalar queue, loads on sync, mask reshapes
   on gpsimd.
 - the emission is software-pipelined (chunk k+1's loads/mean/mask-
   mul before chunk k's broadcast/evict/adds) so the in-order PE and
   DVE queues never stall on each other; small PSUM tiles keep the
   mean ring (2x1 bank) and broadcast ring (3x2 banks) recycling
   faster than the DMA stream.
"""

import sys

sys.path.insert(0, "/opt/trn_rl_repo")

import numpy as np
import ml_dtypes

import concourse.bacc as bacc
import concourse.bass as bass
import concourse.mybir as mybir
import concourse.tile as tile
from concourse import bass_utils

# Problem constants (hardcoded per harness contract)
B, C, H, W = 16, 256, 128, 128
N_BOX = 320
N_CORES = 8
B_SHARD = B // N_CORES  # 2 images per core
HW = H * W  # 16384
P = 128  # partitions
CHUNK = 2048  # free-dim elements per x tile (16 rows of the image)
N_CHUNK = HW // CHUNK  # 8
SUB = 512  # matmul moving free-dim (one PSUM bank of f32)
N_SUB = CHUNK // SUB  # 4
NBOX_PAD = 384  # 320 boxes padded to 3 tiles of 128
N_BOX_TILES = NBOX_PAD // P  # 3
ALPHA = 1.0

F32 = mybir.dt.float32
BF16 = mybir.dt.bfloat16


def build_nc():
    """Build the per-core Bass graph (SPMD: same graph on all 8 cores)."""
    nc = bacc.Bacc(None, target_bir_lowering=False)

    x = nc.declare_dram_parameter("x", [B_SHARD, C, HW], BF16, isOutput=False)
    boxes = nc.declare_dram_parameter("boxes", [P, 4 * N_BOX_TILES], F32, isOutput=False)
    sel = nc.declare_dram_parameter("sel", [P, 2 * N_BOX_TILES], F32, isOutput=False)
    wsum_d = nc.declare_dram_parameter("wsum", [P, 16], BF16, isOutput=False)
    sel4_d = nc.declare_dram_parameter("sel4", [N_SUB, N_SUB * P], BF16, isOutput=False)
    out = nc.declare_dram_parameter("out", [B_SHARD, C, HW], BF16, isOutput=True)

    with tile.TileContext(nc) as tc:
        with (
            tc.tile_pool(name="const", bufs=1) as constp,
            tc.tile_pool(name="boxp", bufs=1) as boxp,
            tc.tile_pool(name="maskp", bufs=1) as maskp,
            tc.tile_pool(name="xp", bufs=8) as xp,
            tc.tile_pool(name="outp", bufs=6) as outp,
            tc.tile_pool(name="corrp", bufs=6) as corrp,
            tc.tile_pool(name="smallp", bufs=4) as smallp,
            tc.tile_pool(name="rowp", bufs=3) as rowp,
            tc.tile_pool(name="meanp", bufs=2, space=bass.MemorySpace.PSUM) as meanp,
            tc.tile_pool(name="pcp", bufs=3, space=bass.MemorySpace.PSUM) as pcp,
        ):
            # --- box/sel data first: the raster chain gates the pipeline ---
            bxf = boxp.tile([P, 4 * N_BOX_TILES], F32, tag="bxf")
            nc.scalar.dma_start(bxf[:], boxes[:])
            stf = boxp.tile([P, 2 * N_BOX_TILES], F32, tag="stf")
            nc.scalar.dma_start(stf[:], sel[:])

            # --- constants ---
            # column (4s+j) of wsum16 is 1/C iff j==s: the mean matmul for
            # sub s uses slice [:, 4s:4s+4] so sub s's channel-sum lands on
            # psum PARTITION s -> the mask-mul runs 4-lane instead of 1-lane
            wsum16 = constp.tile([P, 16], BF16)
            nc.scalar.dma_start(wsum16[:], wsum_d[:])
            # broadcast selectors: sel4[:, 128s:128(s+1)] is a [4,128]
            # stationary whose row s is all-ones -> K=4 matmul against the
            # [4,512] masked-mean tile broadcasts row s to all partitions
            sel4 = constp.tile([N_SUB, N_SUB * P], BF16)
            nc.scalar.dma_start(sel4[:], sel4_d[:])
            iota_i = constp.tile([P, P], mybir.dt.int32)
            nc.gpsimd.iota(iota_i[:], pattern=[[1, P]], base=0, channel_multiplier=0)
            iota_f = constp.tile([P, P], F32)  # each partition: 0..127 along free
            nc.vector.tensor_copy(iota_f[:], iota_i[:])

            # --- box rasterization (tiny; batched [P,3] across box tiles) ---
            # Boxes arrive field-major [128, 12]: col f*3+t = field f of box
            # n = t*128+p. Per box: vx1m1 = (xc-bw/2)*W - 1, vx2 = (xc+bw/2)*W
            # cols[n,w] = (w > vx1m1) & (w <= vx2)   (== ref's clamped-int test)
            # valid = (#cols>=2) & (#rows>=2)        (== ref's x2>x1 & y2>y1)
            T3 = N_BOX_TILES
            xc, yc = bxf[:, 0:T3], bxf[:, T3 : 2 * T3]
            bw, bh = bxf[:, 2 * T3 : 3 * T3], bxf[:, 3 * T3 : 4 * T3]

            def edge(center, halfsrc, w_scale, bias, tag):
                half = smallp.tile([P, T3], F32, tag=tag + "h")
                nc.vector.tensor_scalar_mul(half[:], halfsrc, 0.5)
                lo = smallp.tile([P, T3], F32, tag=tag + "a")
                nc.vector.tensor_tensor(
                    lo[:], center, half[:],
                    op=mybir.AluOpType.subtract if bias else mybir.AluOpType.add,
                )
                o = smallp.tile([P, T3], F32, tag=tag + "b")
                if bias:
                    nc.vector.tensor_scalar(
                        o[:], lo[:], float(w_scale), -1.0,
                        op0=mybir.AluOpType.mult, op1=mybir.AluOpType.add,
                    )
                else:
                    nc.vector.tensor_scalar_mul(o[:], lo[:], float(w_scale))
                return o

            vx1m1 = edge(xc, bw, W, True, "vx1")
            vx2 = edge(xc, bw, W, False, "vx2")
            vy1m1 = edge(yc, bh, H, True, "vy1")
            vy2 = edge(yc, bh, H, False, "vy2")

            ccnt = smallp.tile([P, T3], F32, tag="ccnt")
            rcnt = smallp.tile([P, T3], F32, tag="rcnt")

            def member(lo_m1, hi, t, cnt, tag):
                g1 = smallp.tile([P, P], F32, tag=tag + "g1")
                nc.vector.tensor_scalar(
                    g1[:], iota_f[:], lo_m1[:, t : t + 1], None,
                    op0=mybir.AluOpType.is_gt,
                )
                g2 = smallp.tile([P, P], F32, tag=tag + "g2")
                nc.vector.tensor_scalar(
                    g2[:], iota_f[:], hi[:, t : t + 1], None,
                    op0=mybir.AluOpType.is_le,
                )
                m = boxp.tile([P, P], F32, tag=tag + "m")
                nc.vector.tensor_mul(m[:], g1[:], g2[:])
                nc.vector.tensor_reduce(
                    cnt[:, t : t + 1], m[:], axis=mybir.AxisListType.X,
                    op=mybir.AluOpType.add,
                )
                return m

            cols_raw = [member(vx1m1, vx2, t, ccnt, f"c{t}") for t in range(T3)]
            rows_raw = [member(vy1m1, vy2, t, rcnt, f"r{t}") for t in range(T3)]

            cok = smallp.tile([P, T3], F32, tag="cok")
            nc.vector.tensor_scalar(
                cok[:], ccnt[:], 1.5, None, op0=mybir.AluOpType.is_ge
            )
            rok = smallp.tile([P, T3], F32, tag="rok")
            nc.vector.tensor_scalar(
                rok[:], rcnt[:], 1.5, None, op0=mybir.AluOpType.is_ge
            )
            vfac = smallp.tile([P, T3], F32, tag="vfac")
            nc.vector.tensor_mul(vfac[:], cok[:], rok[:])

            cols_val = []
            for t in range(T3):
                cv = boxp.tile([P, P], F32, tag=f"cv{t}")
                nc.vector.tensor_scalar(
                    cv[:], cols_raw[t][:], vfac[:, t : t + 1], None,
                    op0=mybir.AluOpType.mult,
                )
                cols_val.append(cv)
            rows_sel = [[None] * T3 for _ in range(B_SHARD)]

            # --- pre-emit chunk 0/1 loads + channel-sums: PE starts the
            # moment x lands instead of waiting for the DVE raster chain ---
            rows_per_chunk = CHUNK // W  # 16 image rows per chunk
            chunks = [(b, ci) for b in range(B_SHARD) for ci in range(N_CHUNK)]

            def frontA(b, ci):
                """Loads + channel-sum into psum partitions 0..3."""
                csl = slice(ci * CHUNK, (ci + 1) * CHUNK)
                A = xp.tile([P, CHUNK], BF16, tag="A")
                nc.sync.dma_start(A[:], x[b, 0:P, csl])
                Bt = xp.tile([P, CHUNK], BF16, tag="B")
                nc.sync.dma_start(Bt[:], x[b, P:C, csl])
                pmean = meanp.tile([N_SUB, SUB], F32, tag="pmean")
                for s in range(N_SUB):
                    ssl = slice(s * SUB, (s + 1) * SUB)
                    nc.tensor.matmul(
                        pmean[:], wsum16[:, 4 * s : 4 * s + 4], A[:, ssl],
                        start=(s == 0), stop=False,
                    )
                    nc.tensor.matmul(
                        pmean[:], wsum16[:, 4 * s : 4 * s + 4], Bt[:, ssl],
                        start=False, stop=(s == N_SUB - 1),
                    )
                return (b, ci, csl, A, Bt, pmean)

            stA = {0: frontA(*chunks[0]), 1: frontA(*chunks[1])}

            # --- per-image mask: psum[h,w] = sum_n rows[n,h]*cols[n,w]; clamp. ---
            # Image 0's selector products and mask come first so the main
            # stream's critical path unblocks as early as possible.
            masks = []
            for j in range(B_SHARD):
                for t in range(T3):
                    rs = boxp.tile([P, P], F32, tag=f"rs{t}_{j}")
                    nc.vector.tensor_scalar(
                        rs[:], rows_raw[t][:], stf[:, j * T3 + t : j * T3 + t + 1],
                        None, op0=mybir.AluOpType.mult,
                    )
                    rows_sel[j][t] = rs
                pm = pcp.tile([P, CHUNK // 2], F32, tag="pc")
                for t in range(T3):
                    nc.tensor.matmul(
                        pm[:, 0:W], rows_sel[j][t][:], cols_val[t][:],
                        start=(t == 0), stop=(t == T3 - 1),
                    )
                msb = maskp.tile([P, W], BF16, tag=f"msb{j}")
                nc.vector.tensor_scalar_min(msb[:], pm[:, 0:W], 1.0)
                masks.append(msb)

            # --- main stream: software-pipelined, 2 images x 8 chunks ---
            def frontB(st):
                """Mask reshape + 4-lane mask*mean -> ad (bf16 SBUF)."""
                b, ci, csl, A, Bt, pmean = st
                # this chunk's 16 mask rows [16,128] -> [4,512]: psum
                # partition s holds sub s (4 image rows, 512 px)
                mf = rowp.tile([N_SUB, SUB], BF16, tag="mf")
                nc.gpsimd.dma_start(
                    mf[:],
                    masks[b][ci * rows_per_chunk : (ci + 1) * rows_per_chunk, :],
                )
                ad = rowp.tile([N_SUB, SUB], BF16, tag="ad")
                nc.vector.tensor_mul(ad[:], pmean[:], mf[:])
                return (b, csl, A, Bt, ad)

            def back(st):
                """K=4 broadcast into psum, ACT eviction to bf16, adds, stores."""
                b, csl, A, Bt, ad = st
                corr = corrp.tile([P, CHUNK], BF16, tag="corr")
                for h in range(2):
                    pc = pcp.tile([P, CHUNK // 2], F32, tag="pc")
                    for s2 in range(2):
                        s = 2 * h + s2
                        nc.tensor.matmul(
                            pc[:, s2 * SUB : (s2 + 1) * SUB],
                            sel4[:, s * P : (s + 1) * P], ad[:],
                            start=True, stop=True,
                        )
                    # evict PSUM->SBUF bf16 on the Activation engine so the
                    # DVE adds see all-bf16 SBUF operands (2x DVE mode)
                    nc.scalar.activation(
                        corr[:, h * (CHUNK // 2) : (h + 1) * (CHUNK // 2)],
                        pc[:], mybir.ActivationFunctionType.Copy,
                    )
                oA = outp.tile([P, CHUNK], BF16, tag="oA")
                oB = outp.tile([P, CHUNK], BF16, tag="oB")
                nc.vector.tensor_add(oA[:], A[:], corr[:])
                nc.vector.tensor_add(oB[:], Bt[:], corr[:])
                nc.scalar.dma_start(out[b, 0:P, csl], oA[:])
                nc.scalar.dma_start(out[b, P:C, csl], oB[:])

            stB = {0: frontB(stA[0])}
            for k in range(len(chunks)):
                if k + 2 < len(chunks):
                    stA[k + 2] = frontA(*chunks[k + 2])
                if k + 1 < len(chunks):
                    stB[k + 1] = frontB(stA[k + 1])
                back(stB[k])

    return nc


def _host_prep(x, bboxes, batch_idx):
    """Shard inputs; build padded box/selector arrays (tiny host-side prep)."""
    x = np.ascontiguousarray(np.asarray(x, dtype=np.float32)).reshape(B, C, HW)
    x16 = x.astype(ml_dtypes.bfloat16)
    bboxes = np.asarray(bboxes, dtype=np.float32)
    batch_idx = np.asarray(batch_idx).astype(np.int64)

    boxes_pad = np.zeros((NBOX_PAD, 4), dtype=np.float32)
    boxes_pad[:N_BOX] = bboxes
    # field-major [128, 12]: col f*3+t = field f of box t*128+p
    boxes_fm = np.zeros((P, 4 * N_BOX_TILES), dtype=np.float32)
    for f in range(4):
        for t in range(N_BOX_TILES):
            boxes_fm[:, f * N_BOX_TILES + t] = boxes_pad[t * P : (t + 1) * P, f]

    w16 = np.zeros((P, 16), dtype=ml_dtypes.bfloat16)
    for s in range(4):
        w16[:, 4 * s + s] = ALPHA / C
    s4 = np.zeros((4, 4 * P), dtype=ml_dtypes.bfloat16)
    for s in range(4):
        s4[s, s * P : (s + 1) * P] = 1.0

    in_maps = []
    for i in range(N_CORES):
        sel_i = np.zeros((P, 2 * N_BOX_TILES), dtype=np.float32)
        for j in range(B_SHARD):
            on = np.zeros(NBOX_PAD, dtype=np.float32)
            on[:N_BOX] = (batch_idx == (i * B_SHARD + j)).astype(np.float32)
            for t in range(N_BOX_TILES):
                sel_i[:, j * N_BOX_TILES + t] = on[t * P : (t + 1) * P]
        in_maps.append(
            {
                "x": np.ascontiguousarray(x16[i * B_SHARD : (i + 1) * B_SHARD]),
                "boxes": boxes_fm,
                "sel": sel_i,
                "wsum": w16,
                "sel4": s4,
            }
        )
    return in_maps


def kernel(x, bboxes, batch_idx):
    in_maps = _host_prep(x, bboxes, batch_idx)
    nc = build_nc()
    nc.finalize()
    res = bass_utils.run_bass_kernel_spmd(nc, in_maps, core_ids=list(range(N_CORES)))
    shards = [res.results[i]["out"] for i in range(N_CORES)]
    return (
        np.concatenate(shards, axis=0).astype(np.float32).reshape(B, C, H, W)
    )


if __name__ == "__main__":
    nc = build_nc()
    nc.finalize()
    print("built ok:", len(nc.inst_map), "instructions")
